# revision 1
# baseline (speedup 1.0000x reference)
"""Trainium2 Bass kernel for nn_GameCraftVAEAttention.

Reference computation (B=2, S=4096, C=512, H=8 heads, D=64, GroupNorm G=32):
    x = group_norm(hidden_states)            # stats over (S, 16ch) per group
    q,k,v = x@wq+bq, x@wk+bk, x@wv+bv        # [B,S,512] -> heads [B,S,8,64]
    attn = softmax(q k^T / 8) v              # per (b,h)
    out = attn@wo + bo + hidden_states

Sharding: 16 (batch, head) pairs -> 8 cores, 2 heads (one batch) per core.
Core c: batch b=c//4, heads (2p, 2p+1) with p=c%4.  Each core computes
group-norm for its batch (replicated 4x, cheap), projections for its two
heads, attention, and a partial output projection partial^T = wo_h^T @ o^T.
Host unshard: out[b] = sum_partials^T + bo + residual.

On-core dataflow (everything transposed: channels/head-dim on partitions):
    x[b] --cast bf16--> scratch DRAM --DMA-transpose--> xbT [4x128, 4096]
    stats via DVE free-axis reduces + tiny selector matmuls -> per-channel
    scale/bias -> xnT bf16.  qT/kT/vT = w^T @ xnT (PE).  v transposed back
    via PE to [j, 64|ones] tiles.  Attention per s-chunk of 1024:
      scoresT[j-block, s] = kT^T-slice @ qT  (per head, PSUM)
      expT = Exp(scoresT) on ACT (no max subtraction: |scores| < ~2)
      o^T[65, s] += [v|1]^T @ expT  (PSUM accumulate over j; row 64 = rowsum)
    normalize by rowsum (DVE recip + K=1 ones-matmul broadcast), then
    partial^T = wo_slice^T @ o^T -> DMA out.
"""

import os
import sys

import numpy as np

sys.path.insert(0, "/opt/trn_rl_repo")

import concourse.bacc as bacc
import concourse.bass as bass
import concourse.mybir as mybir
import concourse.tile as tile
from concourse.bass_utils import run_bass_kernel_spmd

B, S, C = 2, 4096, 512
H, D = 8, 64
G = 32
EPS = 1e-6
N_CORES = 8
HPC = 2          # heads per core
D2 = HPC * D     # 128, stacked head dim
CP = 128         # channels per c-tile
NCT = C // CP    # 4 c-tiles
SCHUNK = 1024    # attention s-chunk
NSC = S // SCHUNK
JB = 128         # j block
NJB = S // JB
GPT = CP // (C // G)  # groups per c-tile = 8
CPG = C // G          # channels per group = 16

f32 = mybir.dt.float32
bf16 = mybir.dt.bfloat16
ts = bass.ts


def _body(ctx, tc):
    nc = tc.nc
    AF = mybir.ActivationFunctionType
    OP = mybir.AluOpType

    x_d = nc.dram_tensor("x", [S, C], f32, kind="ExternalInput").ap()
    wq_d = nc.dram_tensor("wq", [C, D2], f32, kind="ExternalInput").ap()
    wk_d = nc.dram_tensor("wk", [C, D2], f32, kind="ExternalInput").ap()
    wv_d = nc.dram_tensor("wv", [C, D2], f32, kind="ExternalInput").ap()
    wo_d = nc.dram_tensor("wo", [D2, C], f32, kind="ExternalInput").ap()
    bq_d = nc.dram_tensor("bq", [D2, 1], f32, kind="ExternalInput").ap()
    bk_d = nc.dram_tensor("bk", [D2, 1], f32, kind="ExternalInput").ap()
    bv_d = nc.dram_tensor("bv", [D2, 1], f32, kind="ExternalInput").ap()
    gnw_d = nc.dram_tensor("gnw", [C], f32, kind="ExternalInput").ap()
    gnb_d = nc.dram_tensor("gnb", [C], f32, kind="ExternalInput").ap()
    selg_d = nc.dram_tensor("selg", [CP, GPT], f32, kind="ExternalInput").ap()
    selb_d = nc.dram_tensor("selb", [GPT, CP], f32, kind="ExternalInput").ap()
    ident_d = nc.dram_tensor("ident", [D, D], bf16, kind="ExternalInput").ap()
    ones_d = nc.dram_tensor("ones", [1, D], f32, kind="ExternalInput").ap()
    pT_d = nc.dram_tensor("pT", [C, S], f32, kind="ExternalOutput").ap()
    xbf_d = nc.dram_tensor("xbf", [NCT, S, CP], bf16).ap()  # internal scratch

    # ---- persistent pools ----
    const_p = ctx.enter_context(tc.tile_pool(name="const", bufs=1))
    xbT_p = ctx.enter_context(tc.tile_pool(name="xbT", bufs=1))
    xnT_p = ctx.enter_context(tc.tile_pool(name="xnT", bufs=1))
    qkv_p = ctx.enter_context(tc.tile_pool(name="qkv", bufs=1))
    vaug_p = ctx.enter_context(tc.tile_pool(name="vaug", bufs=1))
    oT_p = ctx.enter_context(tc.tile_pool(name="oT", bufs=1))

    # ---- constants / weights into SBUF ----
    selg = const_p.tile([CP, GPT], f32)
    nc.sync.dma_start(selg[:], selg_d)
    selb = const_p.tile([GPT, CP], f32)
    nc.sync.dma_start(selb[:], selb_d)
    ident = const_p.tile([D, D], bf16)
    nc.sync.dma_start(ident[:], ident_d)
    ones = const_p.tile([1, D], f32)
    nc.sync.dma_start(ones[:], ones_d)

    w_sb = {}
    for name, wd in (("wq", wq_d), ("wk", wk_d), ("wv", wv_d)):
        t = const_p.tile([CP, NCT, D2], bf16, name=f"w_{name}", tag=f"w_{name}")
        nc.gpsimd.dma_start(t[:], wd.rearrange("(t p) d -> p t d", p=CP))
        w_sb[name] = t
    wo_sb = const_p.tile([D2, C], bf16)
    nc.gpsimd.dma_start(wo_sb[:], wo_d)
    b_sb = {}
    for name, bd in (("bq", bq_d), ("bk", bk_d)):
        t = const_p.tile([D2, 1], f32, name=f"b_{name}", tag=f"b_{name}")
        nc.sync.dma_start(t[:], bd)
        b_sb[name] = t
    bv_sb = const_p.tile([D, HPC], f32)
    nc.sync.dma_start(bv_sb[:], bv_d.rearrange("(h p) o -> p (h o)", p=D))
    gnw = const_p.tile([CP, NCT], f32)
    nc.sync.dma_start(gnw[:], gnw_d.rearrange("(t p) -> p t", p=CP))
    gnb = const_p.tile([CP, NCT], f32)
    nc.sync.dma_start(gnb[:], gnb_d.rearrange("(t p) -> p t", p=CP))

    # ---- phase A: x --cast+split--> bf16 scratch [NCT,S,CP] --transpose--> xbT ----
    xbT = [xbT_p.tile([CP, S], bf16, tag=f"xbT{t}", name=f"xbT{t}") for t in range(NCT)]
    with tc.tile_pool(name="xa", bufs=4) as xa_p:
        for st in range(S // CP):
            xb = xa_p.tile([CP, C], bf16)
            nc.gpsimd.dma_start(xb[:], x_d[st * CP : (st + 1) * CP, :])  # f32->bf16
            for ct in range(NCT):
                nc.sync.dma_start(
                    xbf_d[ct][st * CP : (st + 1) * CP, :], xb[:, ts(ct, CP)]
                )
    for t in range(NCT):
        nc.sync.dma_start(xbT[t][:], xbf_d[t], transpose=True)

    if os.environ.get("KERNEL_PHASES") == "A":
        for t in range(NCT):
            nc.gpsimd.dma_start(pT_d.rearrange("(a p) s -> a p s", p=CP)[t], xbT[t][:])
        return

    # ---- phase B/C/D: group-norm stats -> xnT ----
    xnT = [xnT_p.tile([CP, S], bf16, tag=f"xnT{t}", name=f"xnT{t}") for t in range(NCT)]
    with tc.tile_pool(name="gn_sc", bufs=2) as sq_p, \
         tc.tile_pool(name="gn_st", bufs=1) as st_p, \
         tc.tile_pool(name="gn_ps", bufs=2, space="PSUM") as gps_p:
        st = st_p.tile([CP, 2 * NCT], f32)
        for t in range(NCT):
            nc.vector.reduce_sum(st[:, t : t + 1], xbT[t][:], axis=mybir.AxisListType.X)
            sq = sq_p.tile([CP, S], f32)
            nc.vector.tensor_tensor(sq[:], xbT[t][:], xbT[t][:], op=OP.mult)
            nc.vector.reduce_sum(
                st[:, NCT + t : NCT + t + 1], sq[:], axis=mybir.AxisListType.X
            )
        gst_ps = gps_p.tile([GPT, 2 * NCT], f32)
        nc.tensor.matmul(gst_ps[:], lhsT=selg[:], rhs=st[:], start=True, stop=True)
        # tiny group-stat math on [8, NCT]
        gm = st_p.tile([GPT, 2 * NCT], f32)  # cols 0:4 mean, 4:8 rstd
        inv_n = 1.0 / (CPG * S)
        nc.vector.tensor_scalar_mul(gm[:, 0:NCT], gst_ps[:, 0:NCT], inv_n)
        ex2 = st_p.tile([GPT, NCT], f32)
        nc.vector.tensor_scalar_mul(ex2[:], gst_ps[:, NCT:], inv_n)
        var = st_p.tile([GPT, NCT], f32)
        nc.vector.tensor_tensor(var[:], gm[:, 0:NCT], gm[:, 0:NCT], op=OP.mult)
        nc.vector.tensor_tensor(var[:], ex2[:], var[:], op=OP.subtract)
        eps_t = st_p.tile([GPT, 1], f32)
        nc.vector.memset(eps_t[:], EPS)
        lnv = st_p.tile([GPT, NCT], f32)
        nc.scalar.activation(lnv[:], var[:], AF.Ln, bias=eps_t[:])
        nc.scalar.activation(gm[:, NCT:], lnv[:], AF.Exp, scale=-0.5)

        for t in range(NCT):
            bcm_ps = gps_p.tile([CP, 1], f32, tag="bc")
            nc.tensor.matmul(bcm_ps[:], lhsT=selb[:], rhs=gm[:, t : t + 1], start=True, stop=True)
            bcr_ps = gps_p.tile([CP, 1], f32, tag="bc")
            nc.tensor.matmul(bcr_ps[:], lhsT=selb[:], rhs=gm[:, NCT + t : NCT + t + 1], start=True, stop=True)
            scale_t = st_p.tile([CP, 1], f32, tag=f"sc{t}")
            nc.vector.tensor_tensor(scale_t[:], bcr_ps[:], gnw[:, t : t + 1], op=OP.mult)
            bias_t = st_p.tile([CP, 1], f32, tag=f"bi{t}")
            nc.vector.tensor_tensor(bias_t[:], bcm_ps[:], scale_t[:], op=OP.mult)
            nc.vector.tensor_tensor(bias_t[:], gnb[:, t : t + 1], bias_t[:], op=OP.subtract)
            nc.vector.tensor_scalar(
                xnT[t][:], xbT[t][:], scale_t[:], bias_t[:], op0=OP.mult, op1=OP.add
            )

    if os.environ.get("KERNEL_PHASES") == "D":
        for t in range(NCT):
            nc.gpsimd.dma_start(pT_d.rearrange("(a p) s -> a p s", p=CP)[t], xnT[t][:])
        return

    # ---- phase E: projections qT/kT/vT = w^T @ xnT  ([128, 4096] bf16) ----
    qT = qkv_p.tile([D2, S], bf16)
    kT = qkv_p.tile([D2, S], bf16)
    vTh = [qkv_p.tile([D, S], bf16, name=f"vTh{h}") for h in range(HPC)]
    with tc.tile_pool(name="proj_ps", bufs=3, space="PSUM") as pps:
        for wname, dst, bias, post in (
            ("wq", qT, b_sb["bq"], None),
            ("wk", kT, b_sb["bk"], 0.125),
        ):
            w = w_sb[wname]
            for n in range(S // 512):
                ps = pps.tile([D2, 512], f32)
                for ct in range(NCT):
                    nc.tensor.matmul(
                        ps[:],
                        lhsT=w[:, ct, :],
                        rhs=xnT[ct][:, ts(n, 512)],
                        start=(ct == 0),
                        stop=(ct == NCT - 1),
                    )
                if post is None:
                    nc.vector.tensor_scalar_add(dst[:, ts(n, 512)], ps[:], bias[:])
                else:
                    nc.vector.tensor_scalar(
                        dst[:, ts(n, 512)], ps[:], bias[:], post, op0=OP.add, op1=OP.mult
                    )
        # v: two per-head M=64 chains so vTh tiles sit at base partition 0
        wv = w_sb["wv"]
        for h in range(HPC):
            for n in range(S // 512):
                ps = pps.tile([D, 512], f32, tag="vps")
                for ct in range(NCT):
                    nc.tensor.matmul(
                        ps[:],
                        lhsT=wv[:, ct, h * D : (h + 1) * D],
                        rhs=xnT[ct][:, ts(n, 512)],
                        start=(ct == 0),
                        stop=(ct == NCT - 1),
                    )
                nc.vector.tensor_scalar_add(
                    vTh[h][:, ts(n, 512)], ps[:], bv_sb[:, h : h + 1]
                )

    # ---- phase F: vaug[j-tile] = [v_h0 | 1 | v_h1 | 1]  ([128, 130] bf16) ----
    vaug = [vaug_p.tile([JB, 2 * (D + 1)], bf16, tag=f"va{t}", name=f"va{t}") for t in range(NJB)]
    with tc.tile_pool(name="tp_ps", bufs=4, space="PSUM") as tps:
        for t in range(NJB):
            for h in range(HPC):
                tp = tps.tile([JB, D], bf16)
                nc.tensor.transpose(tp[:], vTh[h][:, ts(t, JB)], ident[:])
                nc.vector.tensor_copy(
                    vaug[t][:, h * (D + 1) : h * (D + 1) + D], tp[:]
                )
            nc.vector.memset(vaug[t][:, D : D + 1], 1.0)
            nc.vector.memset(vaug[t][:, 2 * D + 1 : 2 * D + 2], 1.0)

    if os.environ.get("KERNEL_PHASES") == "F":
        # debug bisect: dump qT/kT and first vaug tiles, skip attention/wo
        nc.gpsimd.dma_start(pT_d.rearrange("(a p) s -> a p s", p=CP)[0], qT[:])
        nc.gpsimd.dma_start(pT_d.rearrange("(a p) s -> a p s", p=CP)[1], kT[:])
        for t in range(8):
            nc.gpsimd.dma_start(
                pT_d.rearrange("(a p) s -> a p s", p=CP)[2][:, t * 130 : t * 130 + 130],
                vaug[t][:],
            )
        return

    # ---- phase G: attention ----
    oT = oT_p.tile([D2, S], bf16)
    with tc.tile_pool(name="sc_ps", bufs=2, space="PSUM") as sps, \
         tc.tile_pool(name="o_ps", bufs=1, space="PSUM") as ops, \
         tc.tile_pool(name="ex_sb", bufs=4) as exp_p, \
         tc.tile_pool(name="nrm_sb", bufs=4) as nrm_p:
        for sc in range(NSC):
            o_ps = [ops.tile([D + 1, SCHUNK], f32, tag=f"o{h}", name=f"ops_{sc}_{h}") for h in range(HPC)]
            for j in range(NJB):
                for h in range(HPC):
                    ps = sps.tile([JB, SCHUNK], f32)
                    for n2 in range(SCHUNK // 512):
                        nc.tensor.matmul(
                            ps[:, ts(n2, 512)],
                            lhsT=kT[h * D : (h + 1) * D, ts(j, JB)],
                            rhs=qT[h * D : (h + 1) * D, sc * SCHUNK + n2 * 512 : sc * SCHUNK + (n2 + 1) * 512],
                            start=True,
                            stop=True,
                        )
                    ex = exp_p.tile([JB, SCHUNK], bf16)
                    nc.scalar.activation(ex[:], ps[:], AF.Exp)
                    for n2 in range(SCHUNK // 512):
                        nc.tensor.matmul(
                            o_ps[h][:, ts(n2, 512)],
                            lhsT=vaug[j][:, h * (D + 1) : (h + 1) * (D + 1)],
                            rhs=ex[:, ts(n2, 512)],
                            start=(j == 0),
                            stop=(j == NJB - 1),
                        )
            for h in range(HPC):
                lnr = nrm_p.tile([1, SCHUNK], f32, tag="lnr")
                nc.scalar.activation(lnr[:], o_ps[h][D : D + 1, :], AF.Ln)
                rec = nrm_p.tile([1, SCHUNK], f32, tag="rec")
                nc.scalar.activation(rec[:], lnr[:], AF.Exp, scale=-1.0)
                bc = ops.tile([D, SCHUNK], f32, tag="o0", name=f"bc_{sc}_{h}")
                for n2 in range(SCHUNK // 512):
                    nc.tensor.matmul(
                        bc[:, ts(n2, 512)],
                        lhsT=ones[:],
                        rhs=rec[:, ts(n2, 512)],
                        start=True,
                        stop=True,
                    )
                o_f = nrm_p.tile([D, SCHUNK], f32, tag="of")
                nc.vector.tensor_copy(o_f[:], o_ps[h][0:D, :])
                nc.vector.tensor_tensor(
                    oT[h * D : (h + 1) * D, ts(sc, SCHUNK)], o_f[:], bc[:], op=OP.mult
                )

    # ---- phase H: partial^T = wo_slice^T @ oT -> DRAM ----
    pT_v = pT_d.rearrange("(t p) s -> t p s", p=CP)
    with tc.tile_pool(name="wo_ps", bufs=3, space="PSUM") as wps, \
         tc.tile_pool(name="wo_sb2", bufs=3) as wsb:
        for cc in range(NCT):
            for n in range(S // 512):
                ps = wps.tile([CP, 512], f32)
                nc.tensor.matmul(
                    ps[:],
                    lhsT=wo_sb[:, ts(cc, CP)],
                    rhs=oT[:, ts(n, 512)],
                    start=True,
                    stop=True,
                )
                ot = wsb.tile([CP, 512], f32)
                nc.vector.tensor_copy(ot[:], ps[:])
                nc.sync.dma_start(pT_v[cc][:, ts(n, 512)], ot[:])


_CACHE = {}


def _build():
    if "nc" in _CACHE:
        return _CACHE["nc"]
    import contextlib

    nc = bacc.Bacc("TRN2", target_bir_lowering=False, debug=False, enable_asserts=False)
    with tile.TileContext(nc) as tc:
        with contextlib.ExitStack() as ctx:
            _body(ctx, tc)
    nc.compile()
    _CACHE["nc"] = nc
    return nc


def _in_maps(inputs):
    x = np.ascontiguousarray(np.asarray(inputs["hidden_states"], dtype=np.float32))
    selg = (np.arange(CP)[:, None] // CPG == np.arange(GPT)[None, :]).astype(np.float32)
    selb = np.ascontiguousarray(selg.T)
    ident = np.eye(D, dtype=np.float32).astype(mybir.dt.np(bf16))
    ones = np.ones((1, D), dtype=np.float32)
    maps = []
    for c in range(N_CORES):
        b = c // (N_CORES // B)
        p = c % (N_CORES // B)
        sl = slice(p * D2, (p + 1) * D2)
        maps.append(
            {
                "x": x[b],
                "wq": np.ascontiguousarray(np.asarray(inputs["wq"], np.float32)[:, sl]),
                "wk": np.ascontiguousarray(np.asarray(inputs["wk"], np.float32)[:, sl]),
                "wv": np.ascontiguousarray(np.asarray(inputs["wv"], np.float32)[:, sl]),
                "wo": np.ascontiguousarray(np.asarray(inputs["wo"], np.float32)[sl, :]),
                "bq": np.ascontiguousarray(np.asarray(inputs["bq"], np.float32)[sl, None]),
                "bk": np.ascontiguousarray(np.asarray(inputs["bk"], np.float32)[sl, None]),
                "bv": np.ascontiguousarray(np.asarray(inputs["bv"], np.float32)[sl, None]),
                "gnw": np.asarray(inputs["gn_w"], np.float32),
                "gnb": np.asarray(inputs["gn_b"], np.float32),
                "selg": selg,
                "selb": selb,
                "ident": ident,
                "ones": ones,
            }
        )
    return maps


def _assemble(inputs, results):
    x = np.asarray(inputs["hidden_states"], dtype=np.float32)
    bo = np.asarray(inputs["bo"], dtype=np.float32)
    out = np.zeros((B, S, C), dtype=np.float32)
    for c in range(N_CORES):
        b = c // (N_CORES // B)
        out[b] += results[c]["pT"].T
    out += bo
    out += x
    return out


def kernel(**inputs):
    nc = _build()
    maps = _in_maps(inputs)
    res = run_bass_kernel_spmd(nc, maps, list(range(N_CORES)))
    return _assemble(inputs, res.results)


if __name__ == "__main__":
    nc = _build()
    print("built ok;", len(nc.m.functions[0].instructions) if hasattr(nc.m.functions[0], "instructions") else "")



# revision 20
# speedup vs baseline: 1.2289x; 1.2289x over previous
"""Trainium2 Bass kernel for nn_GameCraftVAEAttention (v2, restructured).

Reference computation (B=2, S=4096, C=512, H=8 heads, D=64, GroupNorm G=32):
    x = group_norm(hidden_states)            # stats over (S, 16ch) per group
    q,k,v = x@wq+bq, x@wk+bk, x@wv+bv        # [B,S,512] -> heads [B,S,8,64]
    attn = softmax(q k^T / 8) v              # per (b,h)
    out = attn@wo + bo + hidden_states

Sharding: 16 (batch, head) pairs -> 8 cores, 2 heads (one batch) per core.
Host unshard: out[b] = sum of 4 cores' partial^T + bo + residual.

v2 design notes (vs v1 baseline at 655us):
 - x is DMA'd f32->bf16 and transposed ON CHIP via PE-transpose (no DRAM
   scratch round trip); bn_stats runs per 512-col slice as tiles land.
 - GroupNorm is FOLDED INTO the projection weights: xn@W = x@(scale*W)
   + (bias_ch@W).  scale/bias depend on runtime stats only via a cheap
   [128,4] fold; projections read raw x^T directly.
 - Attention: scores^T per (j-pair, head) -> one [128,2x512] Exp on ACT
   (ACT is the roofline engine: 33.5M exps/core at 0.83ns/elem = 220us),
   then ONE fp8 DoubleRow matmul per (pair, head) for AV (2 j-blocks per
   matmul at 0.5 cyc/row: 4x fewer PE cycles than bf16 per-block).
 - rowsums via augmented-V ones column; normalization uses DVE
   reciprocal + tiny bf16 ones-broadcast matmul (keeps ACT exp-only).
 - Per-sc epilogue is software-pipelined: the first PRE j-pairs of the
   next chunk's scores+exp are emitted before the epilogue so ACT never
   starves on the epilogue's cross-engine round trips.
 - pT output in bf16 (partials ~O(0.3); host sums in f32).
"""

import os
import sys

import numpy as np

sys.path.insert(0, "/opt/trn_rl_repo")

import concourse.bacc as bacc
import concourse.bass as bass
import concourse.mybir as mybir
import concourse.tile as tile
from concourse.bass_utils import run_bass_kernel_spmd

B, S, C = 2, 4096, 512
H, D = 8, 64
G = 32
EPS = 1e-6
N_CORES = 8
HPC = 2          # heads per core
D2 = HPC * D     # 128, stacked head dim
CP = 128         # channels per c-tile
NCT = C // CP    # 4 c-tiles
GPT = CP // (C // G)  # groups per c-tile = 8
CPG = C // G          # channels per group = 16
SC = 512         # query chunk
NSC = S // SC    # 8 chunks
NP = 16          # j-pairs (each pair = 2 blocks of 128 keys)
PRE = 3          # j-pairs of next chunk emitted before each epilogue

f32 = mybir.dt.float32
bf16 = mybir.dt.bfloat16
fp8 = mybir.dt.float8e4
ts = bass.ts


def _body(ctx, tc):
    nc = tc.nc
    AF = mybir.ActivationFunctionType
    OP = mybir.AluOpType
    DR = mybir.MatmulPerfMode.DoubleRow

    x_d = nc.dram_tensor("x", [S, C], f32, kind="ExternalInput").ap()
    wq_d = nc.dram_tensor("wq", [C, D2], f32, kind="ExternalInput").ap()
    wk_d = nc.dram_tensor("wk", [C, D2], f32, kind="ExternalInput").ap()
    wv_d = nc.dram_tensor("wv", [C, D2], f32, kind="ExternalInput").ap()
    wo_d = nc.dram_tensor("wo", [D2, C], f32, kind="ExternalInput").ap()
    bq_d = nc.dram_tensor("bq", [D2, 1], f32, kind="ExternalInput").ap()
    bk_d = nc.dram_tensor("bk", [D2, 1], f32, kind="ExternalInput").ap()
    bv_d = nc.dram_tensor("bv", [D2, 1], f32, kind="ExternalInput").ap()
    gnw_d = nc.dram_tensor("gnw", [C], f32, kind="ExternalInput").ap()
    gnb_d = nc.dram_tensor("gnb", [C], f32, kind="ExternalInput").ap()
    selg_d = nc.dram_tensor("selg", [CP, GPT], f32, kind="ExternalInput").ap()
    selb_d = nc.dram_tensor("selb", [GPT, CP], f32, kind="ExternalInput").ap()
    id128_d = nc.dram_tensor("id128", [CP, CP], bf16, kind="ExternalInput").ap()
    id64_d = nc.dram_tensor("id64", [2 * D, D], bf16, kind="ExternalInput").ap()
    ones64_d = nc.dram_tensor("ones64", [1, D], bf16, kind="ExternalInput").ap()
    pT_d = nc.dram_tensor("pT", [C, S], bf16, kind="ExternalOutput").ap()
    pT_v = pT_d.rearrange("(t p) s -> t p s", p=CP)

    # ---- persistent pools ----
    const_p = ctx.enter_context(tc.tile_pool(name="const", bufs=1))
    xbT_p = ctx.enter_context(tc.tile_pool(name="xbT", bufs=1))
    qkv_p = ctx.enter_context(tc.tile_pool(name="qkv", bufs=1))
    vaug_p = ctx.enter_context(tc.tile_pool(name="vaug", bufs=1))
    stat_p = ctx.enter_context(tc.tile_pool(name="stat", bufs=1))

    # ---- constants / weights into SBUF ----
    selg = const_p.tile([CP, GPT], f32)
    nc.sync.dma_start(selg[:], selg_d)
    selb = const_p.tile([GPT, CP], f32)
    nc.sync.dma_start(selb[:], selb_d)
    id128 = const_p.tile([CP, CP], bf16)
    nc.sync.dma_start(id128[:], id128_d)
    id64 = const_p.tile([2 * D, D], bf16)
    nc.sync.dma_start(id64[:], id64_d)
    ones64 = const_p.tile([1, D], bf16)
    nc.sync.dma_start(ones64[:], ones64_d)

    w_sb = {}
    for name, wd in (("wq", wq_d), ("wk", wk_d), ("wv", wv_d)):
        t = const_p.tile([CP, NCT, D2], bf16, name=f"w_{name}", tag=f"w_{name}")
        nc.gpsimd.dma_start(t[:], wd.rearrange("(t p) d -> p t d", p=CP))
        w_sb[name] = t
    wo_sb = const_p.tile([D2, C], bf16)
    nc.gpsimd.dma_start(wo_sb[:], wo_d)
    b_sb = {}
    for name, bd in (("bq", bq_d), ("bk", bk_d), ("bv", bv_d)):
        t = const_p.tile([D2, 1], f32, name=f"b_{name}", tag=f"b_{name}")
        nc.sync.dma_start(t[:], bd)
        b_sb[name] = t
    gnw = const_p.tile([CP, NCT], f32)
    nc.sync.dma_start(gnw[:], gnw_d.rearrange("(t p) -> p t", p=CP))
    gnb = const_p.tile([CP, NCT], f32)
    nc.sync.dma_start(gnb[:], gnb_d.rearrange("(t p) -> p t", p=CP))

    # ---- phase A: x --DMA cast+PE transpose--> xbT[4] [128, S] bf16,
    #      with per-slice bn_stats as slices land ----
    xbT = [xbT_p.tile([CP, S], bf16, tag=f"xbT{t}", name=f"xbT{t}") for t in range(NCT)]
    st6 = [stat_p.tile([CP, 8, 6], f32, tag=f"st{t}", name=f"st{t}") for t in range(NCT)]
    with tc.tile_pool(name="xa", bufs=2) as xa_p, \
         tc.tile_pool(name="tps", bufs=4, space="PSUM") as tps_p:
        for sg in range(8):
            xbs = []
            for k4 in range(4):
                xb = xa_p.tile([CP, C], bf16, tag=f"x{k4}", name=f"x_{sg}_{k4}")
                nc.gpsimd.dma_start(
                    xb[:], x_d[(4 * sg + k4) * CP : (4 * sg + k4 + 1) * CP, :]
                )
                xbs.append(xb)
            for ct in range(NCT):
                tp4 = tps_p.tile([CP, 4 * CP], bf16)
                for k4 in range(4):
                    nc.tensor.transpose(
                        tp4[:, ts(k4, CP)], xbs[k4][:, ts(ct, CP)], id128[:]
                    )
                nc.vector.tensor_copy(xbT[ct][:, ts(sg, 512)], tp4[:])
                nc.vector.bn_stats(st6[ct][:, sg, :], xbT[ct][:, ts(sg, 512)])

    if os.environ.get("KERNEL_PHASES") == "A":
        for t in range(NCT):
            nc.gpsimd.dma_start(pT_v[t], xbT[t][:])
        return

    # ---- phase B: combine stats -> per-channel scale/bias ----
    mv = stat_p.tile([CP, NCT, 2], f32)        # per-channel (mean, var)
    stats8 = stat_p.tile([CP, 2 * NCT], f32)   # cols 0:4 mean, 4:8 E[x^2]
    scale = stat_p.tile([CP, NCT], f32)
    biasch = stat_p.tile([CP, NCT], f32)
    biasch_bf = stat_p.tile([CP, NCT], bf16)
    with tc.tile_pool(name="gps", bufs=2, space="PSUM") as gps_p:
        for ct in range(NCT):
            nc.vector.bn_aggr(mv[:, ct, :], st6[ct][:])
        nc.vector.tensor_copy(stats8[:, 0:NCT], mv[:, :, 0])
        nc.vector.tensor_tensor(stats8[:, NCT:], mv[:, :, 0], mv[:, :, 0], op=OP.mult)
        nc.vector.tensor_tensor(stats8[:, NCT:], stats8[:, NCT:], mv[:, :, 1], op=OP.add)
        gsum = gps_p.tile([GPT, 2 * NCT], f32)
        nc.tensor.matmul(gsum[:], lhsT=selg[:], rhs=stats8[:], start=True, stop=True)
        gm8 = stat_p.tile([GPT, 2 * NCT], f32)  # cols 0:4 gmean, 4:8 -> rstd
        nc.vector.tensor_scalar_mul(gm8[:], gsum[:], 1.0 / CPG)
        gvar = stat_p.tile([GPT, NCT], f32)
        nc.vector.tensor_tensor(gvar[:], gm8[:, 0:NCT], gm8[:, 0:NCT], op=OP.mult)
        nc.vector.tensor_tensor(gvar[:], gm8[:, NCT:], gvar[:], op=OP.subtract)
        eps_t = stat_p.tile([GPT, 1], f32)
        nc.vector.memset(eps_t[:], EPS)
        gsd = stat_p.tile([GPT, NCT], f32)
        nc.scalar.activation(gsd[:], gvar[:], AF.Sqrt, bias=eps_t[:])
        nc.vector.reciprocal(gm8[:, NCT:], gsd[:])
        bcast = gps_p.tile([CP, 2 * NCT], f32)
        nc.tensor.matmul(bcast[:], lhsT=selb[:], rhs=gm8[:], start=True, stop=True)
        nc.vector.tensor_tensor(scale[:], bcast[:, NCT:], gnw[:], op=OP.mult)
        nc.vector.tensor_tensor(biasch[:], bcast[:, 0:NCT], scale[:], op=OP.mult)
        nc.vector.tensor_tensor(biasch[:], gnb[:], biasch[:], op=OP.subtract)
        nc.vector.tensor_copy(biasch_bf[:], biasch[:])

    # ---- phase C: fold scale into weights; bias projections ----
    wsc = {}
    for name in ("wq", "wk", "wv"):
        wsc[name] = const_p.tile([CP, NCT, D2], bf16, name=f"ws_{name}", tag=f"ws_{name}")
    tb = {}
    with tc.tile_pool(name="bps", bufs=2, space="PSUM") as bps_p:
        for ct in range(NCT):
            nc.vector.tensor_scalar(
                wsc["wq"][:, ct, :], w_sb["wq"][:, ct, :],
                scale[:, ct : ct + 1], None, op0=OP.mult,
            )
            nc.vector.tensor_scalar(
                wsc["wk"][:, ct, :], w_sb["wk"][:, ct, :],
                scale[:, ct : ct + 1], 0.125, op0=OP.mult, op1=OP.mult,
            )
            nc.vector.tensor_scalar(
                wsc["wv"][:, ct, :], w_sb["wv"][:, ct, :],
                scale[:, ct : ct + 1], None, op0=OP.mult,
            )
        for name, bias in (("wq", "bq"), ("wk", "bk"), ("wv", "bv")):
            psb = bps_p.tile([D2, 1], f32)
            for ct in range(NCT):
                nc.tensor.matmul(
                    psb[:], lhsT=w_sb[name][:, ct, :], rhs=biasch_bf[:, ct : ct + 1],
                    start=(ct == 0), stop=(ct == NCT - 1),
                )
            t = stat_p.tile([D2, 1], f32, name=f"tb_{name}", tag=f"tb_{name}")
            if name == "wk":
                nc.vector.tensor_scalar(
                    t[:], psb[:], b_sb[bias][:], 0.125, op0=OP.add, op1=OP.mult
                )
            else:
                nc.vector.tensor_tensor(t[:], psb[:], b_sb[bias][:], op=OP.add)
            tb[name] = t

    # ---- phase D: projections (k, v, q) + v transpose into fp8 vaug ----
    qT = qkv_p.tile([D2, S], bf16, tag="qT", name="qT")
    kT = qkv_p.tile([D2, S], bf16, tag="kT", name="kT")
    vT = qkv_p.tile([D2, S], bf16, tag="vT", name="vT")
    # vaug2[p]: [128 (j within block), 2 (block of pair), 160] fp8
    #   cols per head h: [80h : 80h+64] = v^T, [80h+64 : 80h+80] = ones
    #   (DoubleRow stationary M=80: one matmul yields o rows 0:64 AND
    #    rowsum rows 64:80 at dst partition base 0)
    vaug2 = [
        vaug_p.tile([CP, 2, 2 * (D + 16)], fp8, tag=f"va{p}", name=f"va{p}")
        for p in range(NP)
    ]
    with tc.tile_pool(name="pps", bufs=3, space="PSUM") as pps_p, \
         tc.tile_pool(name="tps2", bufs=4, space="PSUM") as tps2_p:
        for dst, wname in ((kT, "wk"), (vT, "wv"), (qT, "wq")):
            ws = wsc[wname]
            for n in range(NSC):
                ps = pps_p.tile([D2, SC], f32)
                for ct in range(NCT):
                    nc.tensor.matmul(
                        ps[:], lhsT=ws[:, ct, :], rhs=xbT[ct][:, ts(n, SC)],
                        start=(ct == 0), stop=(ct == NCT - 1),
                    )
                nc.vector.tensor_scalar_add(dst[:, ts(n, SC)], ps[:], tb[wname][:])
        for p in range(NP):
            for h in range(HPC):
                tp = tps2_p.tile([CP, CP], bf16)
                for j2 in range(2):
                    nc.tensor.transpose(
                        tp[:, ts(j2, D)],
                        vT[h * D : (h + 1) * D, (2 * p + j2) * CP : (2 * p + j2 + 1) * CP],
                        id64[h * D : (h + 1) * D, :],
                    )
                nc.vector.tensor_copy(
                    vaug2[p][:, :, h * (D + 16) : h * (D + 16) + D], tp[:]
                )
                nc.vector.memset(
                    vaug2[p][:, :, h * (D + 16) + D : (h + 1) * (D + 16)], 1.0
                )

    if os.environ.get("KERNEL_PHASES") == "D":
        nc.gpsimd.dma_start(pT_v[0], qT[:])
        nc.gpsimd.dma_start(pT_v[1], kT[:])
        nc.gpsimd.dma_start(pT_v[2], vT[:])
        return

    # ---- phase E: attention, software-pipelined epilogue ----
    with tc.tile_pool(name="sps", bufs=2, space="PSUM") as sps_p, \
         tc.tile_pool(name="ops", bufs=1, space="PSUM") as ops_p, \
         tc.tile_pool(name="bcp", bufs=1, space="PSUM") as bc_p, \
         tc.tile_pool(name="pop", bufs=1, space="PSUM") as po_p, \
         tc.tile_pool(name="exp", bufs=8) as ex_p, \
         tc.tile_pool(name="nrm", bufs=4) as nrm_p, \
         tc.tile_pool(name="ocp", bufs=2) as oc_p, \
         tc.tile_pool(name="pout", bufs=2) as pout_p:

        def emit_scores_exp(sc, p, h):
            ps = sps_p.tile([CP, 2, SC], f32, tag="ps", name=f"ps_{sc}_{p}_{h}")
            for j2 in range(2):
                nc.tensor.matmul(
                    ps[:, j2, :],
                    lhsT=kT[h * D : (h + 1) * D, (2 * p + j2) * CP : (2 * p + j2 + 1) * CP],
                    rhs=qT[h * D : (h + 1) * D, ts(sc, SC)],
                    start=True, stop=True,
                )
            ex = ex_p.tile([CP, 2, SC], fp8, tag="ex", name=f"ex_{sc}_{p}_{h}")
            nc.scalar.activation(ex[:], ps[:], AF.Exp)
            return ex

        def emit_av(o_ps, p, h, ex):
            nc.tensor.matmul(
                o_ps[h][:],
                lhsT=vaug2[p][:, :, h * (D + 16) : (h + 1) * (D + 16)],
                rhs=ex[:],
                start=(p == 0), stop=(p == NP - 1),
                perf_mode=DR,
            )

        def emit_epilogue(sc, o_ps):
            oc = oc_p.tile([D2, SC], bf16, tag="oc", name=f"oc_{sc}")
            for h in range(HPC):
                rec = nrm_p.tile([1, SC], f32, tag="rec", name=f"rec_{sc}_{h}")
                nc.vector.reciprocal(rec[:], o_ps[h][D : D + 1, :])
                rec_bf = nrm_p.tile([1, SC], bf16, tag="recb", name=f"recb_{sc}_{h}")
                nc.vector.tensor_copy(rec_bf[:], rec[:])
                bc = bc_p.tile([D, SC], f32, tag="bc", name=f"bc_{sc}_{h}")
                nc.tensor.matmul(bc[:], lhsT=ones64[:], rhs=rec_bf[:], start=True, stop=True)
                o_sb = nrm_p.tile([D, SC], bf16, tag="osb", name=f"osb_{sc}_{h}")
                nc.vector.tensor_copy(o_sb[:], o_ps[h][0:D, :])
                nc.vector.tensor_tensor(
                    oc[h * D : (h + 1) * D, :], o_sb[:], bc[:], op=OP.mult
                )
            for cc in range(NCT):
                po = po_p.tile([CP, SC], f32, tag="po", name=f"po_{sc}_{cc}")
                nc.tensor.matmul(
                    po[:], lhsT=wo_sb[:, ts(cc, CP)], rhs=oc[:], start=True, stop=True
                )
                pout = pout_p.tile([CP, SC], bf16, tag="pout", name=f"pout_{sc}_{cc}")
                nc.vector.tensor_copy(pout[:], po[:])
                nc.sync.dma_start(pT_v[cc][:, ts(sc, SC)], pout[:])

        exq = {}
        prev_o = None
        for sc in range(NSC):
            o_ps = [
                ops_p.tile([D + 16, SC], f32, tag=f"o{h}", name=f"ops_{sc}_{h}")
                for h in range(HPC)
            ]
            pre = PRE if sc > 0 else 0
            for p in range(pre):
                for h in range(HPC):
                    emit_av(o_ps, p, h, exq.pop((sc, p, h)))
            for p in range(pre, NP):
                for h in range(HPC):
                    ex = emit_scores_exp(sc, p, h)
                    emit_av(o_ps, p, h, ex)
            if sc < NSC - 1:
                for p in range(PRE):
                    for h in range(HPC):
                        exq[(sc + 1, p, h)] = emit_scores_exp(sc + 1, p, h)
            emit_epilogue(sc, o_ps)
            prev_o = o_ps


_CACHE = {}


def _build():
    if "nc" in _CACHE:
        return _CACHE["nc"]
    import contextlib

    nc = bacc.Bacc("TRN2", target_bir_lowering=False, debug=False, enable_asserts=False)
    with tile.TileContext(nc) as tc:
        with contextlib.ExitStack() as ctx:
            _body(ctx, tc)
    nc.compile()
    _CACHE["nc"] = nc
    return nc


def _in_maps(inputs):
    x = np.ascontiguousarray(np.asarray(inputs["hidden_states"], dtype=np.float32))
    selg = (np.arange(CP)[:, None] // CPG == np.arange(GPT)[None, :]).astype(np.float32)
    selb = np.ascontiguousarray(selg.T)
    bfnp = mybir.dt.np(bf16)
    id128 = np.eye(CP, dtype=np.float32).astype(bfnp)
    id64 = np.tile(np.eye(D, dtype=np.float32), (2, 1)).astype(bfnp)
    ones64 = np.ones((1, D), dtype=np.float32).astype(bfnp)
    maps = []
    for c in range(N_CORES):
        b = c // (N_CORES // B)
        p = c % (N_CORES // B)
        sl = slice(p * D2, (p + 1) * D2)
        maps.append(
            {
                "x": x[b],
                "wq": np.ascontiguousarray(np.asarray(inputs["wq"], np.float32)[:, sl]),
                "wk": np.ascontiguousarray(np.asarray(inputs["wk"], np.float32)[:, sl]),
                "wv": np.ascontiguousarray(np.asarray(inputs["wv"], np.float32)[:, sl]),
                "wo": np.ascontiguousarray(np.asarray(inputs["wo"], np.float32)[sl, :]),
                "bq": np.ascontiguousarray(np.asarray(inputs["bq"], np.float32)[sl, None]),
                "bk": np.ascontiguousarray(np.asarray(inputs["bk"], np.float32)[sl, None]),
                "bv": np.ascontiguousarray(np.asarray(inputs["bv"], np.float32)[sl, None]),
                "gnw": np.asarray(inputs["gn_w"], np.float32),
                "gnb": np.asarray(inputs["gn_b"], np.float32),
                "selg": selg,
                "selb": selb,
                "id128": id128,
                "id64": id64,
                "ones64": ones64,
            }
        )
    return maps


def _assemble(inputs, results):
    x = np.asarray(inputs["hidden_states"], dtype=np.float32)
    bo = np.asarray(inputs["bo"], dtype=np.float32)
    out = np.zeros((B, S, C), dtype=np.float32)
    for c in range(N_CORES):
        b = c // (N_CORES // B)
        out[b] += np.asarray(results[c]["pT"], dtype=np.float32).T
    out += bo
    out += x
    return out


def kernel(**inputs):
    nc = _build()
    maps = _in_maps(inputs)
    res = run_bass_kernel_spmd(nc, maps, list(range(N_CORES)))
    return _assemble(inputs, res.results)


if __name__ == "__main__":
    nc = _build()
    print("built ok")
